# revision 6
# baseline (speedup 1.0000x reference)
"""ArgMaxTop Trainium2 kernel (v5 — dual HWDGE queues + balanced tail).

Math: out[b] = argmax_c sum_s x[b,s,c] * [x[b,s,c] >= t(b,s)] where t is the
8th-largest value of row (b,s). Equal to the reference's scatter-top8/mean/
argmax pipeline for inputs without exact float ties.

Sharding: batch b -> core b (8 batches, 8 cores), no collectives.

Per-core dataflow, per 128-row s-tile (16 tiles), chunks of 6400 (5/tile):
  - DMA x chunks [128, 6400] f32 to SBUF (pool of 7). Chunk 0 issues from
    the ACT HWDGE ring, chunks 1-4 from the sync HWDGE ring: a single ring
    serializes each 3.28 MB transfer behind its ~2 us completion receipt
    (measured 9.8 us/chunk = 333 GB/s); a second ring lets the SDMA engines
    interleave packets so the fixed costs overlap. (SWDGE/gpsimd is NOT
    usable here: DVE 2-port perf mode locks GpSimd out of the descriptor
    rings, starving its queue — measured 9.7 us stream holes.) Chunk 0's
    buffer is freed a whole tile earlier, so the ACT-issued DMA never
    blocks ACT's compute stream.
  - DVE max8 per chunk -> concat [128,40] -> max8 -> t (8th largest); all
    per-tile scalar prep (tneg/tpneg/t2col/t2f) also on DVE, keeping ACT
    pure relu/sign
  - value stream r = Relu(x - t) in fp16 (ACT), ones-stationary fp16
  - selection stream, split to balance DVE/ACT:
      * DVE halves: sel = (x >= t) * 2 in {0,2}, fp16-exact (fused
        tensor_scalar 2x); contributes t/2 * 2m = t*m via w2
      * ACT halves: sel = Sign(x - t') in {-1,+1}; t' = t*(1-2^-23) sits
        strictly between the 9th and 8th largest for every row of this
        input; the class-independent -t/2 is added back on the host
        (T_half = per-tile sum of fp16(t/2), acc2 psum column)
  - LAST tile: everything after the final byte is a serial tail, so (a) its
    last chunk loads as two half-chunk DMAs so the final max8 is 3.2k wide,
    and (b) the 20 half-stream units split for latency: DVE (2x
    tensor_scalar) takes all 10 sel halves + 2 relu halves, ACT takes 8
    relu halves -> ~35 us tail instead of ~57. No sign stream in tile 15,
    so the host correction sums only tiles 0..14.
  - PE: per 400-wide class window, two matmuls accumulate into PSUM
    [80,400] via shifted one-hot stationaries
  - drain psum -> SBUF -> DRAM out [80,400] + tsum [1,16]; host adds
    T_half to the sign-path classes (>= 16000) and argmaxes.
"""

import sys

if "/opt/trn_rl_repo" not in sys.path:
    sys.path.insert(0, "/opt/trn_rl_repo")

import numpy as np

B, S, C = 8, 2048, 32000
TOP_K = 8
P = 128            # partitions per s-tile
XCH = 6400         # x chunk width
NXCH = C // XCH    # 5 chunks per tile
CCH = 400          # matmul moving window / psum columns
NW = XCH // CCH    # 16 windows per chunk
NROWS = C // CCH   # 80 psum rows
NTILES = S // P    # 16
HCH = XCH // 2     # stream granularity (half chunk)
# (chunk, half) pairs whose selection runs on DVE in tiles 0..14 (rest: ACT
# Sign); classes below SIGN_CLASS_START never see the sign stream
DVE_SEL_HALVES = {(0, 0), (0, 1), (1, 0), (1, 1), (2, 0)}
SIGN_CLASS_START = 2 * XCH + XCH // 2  # 16000
# relu halves of the LAST tile that run on DVE (ACT takes the rest)
TAIL_DVE_RELU = {(0, 0), (0, 1)}

_CACHE = {}


def _build_graph():
    from concourse import bacc, tile, mybir

    f32 = mybir.dt.float32
    f16 = mybir.dt.float16
    Alu = mybir.AluOpType
    Act = mybir.ActivationFunctionType

    nc = bacc.Bacc(
        "TRN2",
        target_bir_lowering=False,
        debug=False,
        # largest DMA here is 128 descriptors; the 16 KB default wastes SBUF
        dynamic_dma_scratch_size=4096,
    )
    x = nc.dram_tensor("x", [S, C], f32, kind="ExternalInput").ap()
    out = nc.dram_tensor("out", [NROWS, CCH], f32, kind="ExternalOutput").ap()
    tsum = nc.dram_tensor("tsum", [1, NTILES], f32, kind="ExternalOutput").ap()

    n_mm = NTILES * NXCH * NW * 2
    mm_i = 0
    SCALE_P = float(np.float32(1.0) - np.float32(2.0**-23))

    with tile.TileContext(nc) as tc:
        with (
            tc.tile_pool(name="consts", bufs=1) as consts,
            tc.tile_pool(name="xp", bufs=7) as xp,
            tc.tile_pool(name="tp", bufs=2) as tp,
            tc.tile_pool(name="rp", bufs=2) as rp,
            tc.tile_pool(name="selp", bufs=2) as selp,
            tc.tile_pool(name="sump", bufs=1) as sump,
            tc.tile_pool(name="ps", bufs=1, space="PSUM") as ps,
            tc.tile_pool(name="ps2", bufs=1, space="PSUM") as ps2,
        ):
            # ones stationary (relu stream + T_half column): 1.0 at col NROWS
            ztm = consts.tile([P, 2 * NROWS], f16, name="ztm")
            nc.vector.memset(ztm, 0.0)
            nc.vector.memset(ztm[:, NROWS : NROWS + 1], 1.0)
            # t/2 stationaries: zeros except col NROWS, overwritten with
            # fp16(t/2) every tile (parity double-buffered)
            w2 = [
                consts.tile([P, 2 * NROWS], f16, name=f"w2_{k}")
                for k in range(2)
            ]
            for w in w2:
                nc.vector.memset(w, 0.0)

            acc = ps.tile([NROWS, CCH], f32, name="acc")
            acc2 = ps2.tile([1, NTILES], f32, name="acc2")

            for it in range(NTILES):
                last = it == NTILES - 1
                # load chunks; chunk 0 rides the ACT HWDGE ring, the rest
                # the sync ring. The last tile's final chunk is split into
                # two half-chunk DMAs to shorten the final max8 latency.
                xch = []
                xlast = None
                for j in range(NXCH):
                    dma_eng = nc.scalar if j == 0 else nc.sync
                    src = x[it * P : (it + 1) * P, j * XCH : (j + 1) * XCH]
                    if last and j == NXCH - 1:
                        xt = xp.tile([P, XCH], f32, name="xch", tag="xch")
                        nc.sync.dma_start(
                            out=xt[:, :HCH], in_=src[:, :HCH]
                        )
                        nc.sync.dma_start(
                            out=xt[:, HCH:], in_=src[:, HCH:]
                        )
                        xlast = xt
                        xch.append(xt)
                    else:
                        xt = xp.tile([P, XCH], f32, name="xch", tag="xch")
                        dma_eng.dma_start(out=xt, in_=src)
                        xch.append(xt)

                if last:
                    top = tp.tile([P, 8 * NXCH + 8], f32, name="topl", tag="topl")
                    for j in range(NXCH - 1):
                        nc.vector.max(
                            out=top[:, 8 * j : 8 * (j + 1)], in_=xch[j]
                        )
                    nc.vector.max(
                        out=top[:, 8 * (NXCH - 1) : 8 * NXCH],
                        in_=xlast[:, :HCH],
                    )
                    nc.vector.max(
                        out=top[:, 8 * NXCH : 8 * NXCH + 8],
                        in_=xlast[:, HCH:],
                    )
                else:
                    top = tp.tile([P, 8 * NXCH], f32, name="top", tag="top")
                    for j in range(NXCH):
                        nc.vector.max(
                            out=top[:, 8 * j : 8 * (j + 1)], in_=xch[j]
                        )
                top8 = tp.tile([P, 8], f32, name="top8", tag="top8")
                nc.vector.max(out=top8, in_=top)
                t_ap = top8[:, 7:8]

                # per-tile scalars, all on DVE so ACT stays pure relu/sign
                tneg = tp.tile([P, 1], f32, name="tneg", tag="tneg")
                nc.vector.tensor_scalar(tneg, t_ap, -1.0, None, Alu.mult)
                t2col = w2[it % 2][:, NROWS : NROWS + 1]
                nc.vector.tensor_scalar(t2col, t_ap, 0.5, None, Alu.mult)
                if not last:
                    tpneg = tp.tile([P, 1], f32, name="tpneg", tag="tpneg")
                    nc.vector.tensor_scalar(
                        tpneg, t_ap, -SCALE_P, None, Alu.mult
                    )
                # per-tile T_half = sum_s fp16(t_s/2) (own psum column,
                # host-summed over tiles 0..14; reads the ROUNDED fp16 t/2
                # so the host correction matches the stationary exactly)
                t2f = tp.tile([P, 1], f16, name="t2f", tag="t2f")
                nc.vector.tensor_copy(t2f, t2col)
                nc.tensor.matmul(
                    acc2[:, it : it + 1],
                    ztm[:, NROWS : NROWS + 1],
                    t2f,
                    start=True,
                    stop=True,
                )

                for j in range(NXCH):
                    for hh in range(2):
                        xs = xch[j][:, hh * HCH : (hh + 1) * HCH]
                        r = rp.tile([P, HCH], f16, name="r", tag="r")
                        if last and (j, hh) in TAIL_DVE_RELU:
                            nc.vector.tensor_scalar(
                                r, xs, t_ap, 0.0, Alu.subtract, Alu.max
                            )
                        else:
                            nc.scalar.activation(
                                out=r,
                                in_=xs,
                                func=Act.Relu,
                                bias=tneg,
                                scale=1.0,
                            )
                        sel = selp.tile([P, HCH], f16, name="sel", tag="sel")
                        if last or (j, hh) in DVE_SEL_HALVES:
                            nc.vector.tensor_scalar(
                                sel, xs, t_ap, 2.0, Alu.is_ge, Alu.mult
                            )
                        else:
                            nc.scalar.activation(
                                out=sel,
                                in_=xs,
                                func=Act.Sign,
                                bias=tpneg,
                                scale=1.0,
                            )
                        wsel = w2[it % 2]
                        for w in range(HCH // CCH):
                            cg = j * NW + hh * (HCH // CCH) + w
                            off = w * CCH
                            nc.tensor.matmul(
                                acc,
                                ztm[:, NROWS - cg : 2 * NROWS - cg],
                                r[:, off : off + CCH],
                                start=(mm_i == 0),
                                stop=(mm_i == n_mm - 1),
                            )
                            mm_i += 1
                            nc.tensor.matmul(
                                acc,
                                wsel[:, NROWS - cg : 2 * NROWS - cg],
                                sel[:, off : off + CCH],
                                start=False,
                                stop=(mm_i == n_mm - 1),
                            )
                            mm_i += 1

            sums = sump.tile([NROWS, CCH], f32, name="sums")
            nc.vector.tensor_copy(sums, acc)
            nc.sync.dma_start(out=out, in_=sums)
            tsums = sump.tile([1, NTILES], f32, name="tsums")
            nc.vector.tensor_copy(tsums, acc2)
            nc.sync.dma_start(out=tsum, in_=tsums)

    nc.compile()
    return nc


def _in_maps(x):
    return [{"x": np.ascontiguousarray(x[b])} for b in range(B)]


def _postprocess(sums_2d, tsum_row):
    sums = np.asarray(sums_2d, dtype=np.float64).reshape(-1)
    # tile 15 has no sign stream; only tiles 0..14 need the correction
    t_half = float(
        np.asarray(tsum_row, dtype=np.float64).reshape(-1)[: NTILES - 1].sum()
    )
    sums[SIGN_CLASS_START:] += t_half
    return sums


def kernel(**inputs):
    from concourse import bass_utils

    x = np.asarray(inputs["inputs"], dtype=np.float32)
    assert x.shape == (B, S, C), x.shape

    if "nc" not in _CACHE:
        _CACHE["nc"] = _build_graph()
    nc = _CACHE["nc"]

    res = bass_utils.run_bass_kernel_spmd(
        nc, _in_maps(x), core_ids=list(range(B))
    )

    out = np.empty((B,), dtype=np.int32)
    for b in range(B):
        sums = _postprocess(res.results[b]["out"], res.results[b]["tsum"])
        out[b] = np.argmax(sums)
    return out


# revision 12
# speedup vs baseline: 1.4535x; 1.4535x over previous
"""ArgMaxTop Trainium2 kernel (v8 — fine-grained chunks + pool slack).

Math: out[b] = argmax_c sum_s x[b,s,c] * [x[b,s,c] >= t(b,s)] where t is the
8th-largest value of row (b,s). Equal to the reference's scatter-top8/mean/
argmax pipeline for inputs without exact float ties.

Sharding: batch b -> core b (8 batches, 8 cores), no collectives.

Why 3200-wide chunks: the sync HWDGE ring streams back-to-back chunk DMAs
at ~432 GB/s (measured at startup — the ~2 us completion receipt does NOT
serialize the ring), so steady-state DMA pacing is set by BUFFER RELEASES,
not the ring. The recurrence arrival -> max8 -> t -> streams -> release ->
next DMA settled at ~50 us/tile with 6400-wide chunks while the engines
only need ~44 us. Halving the chunk width (13-buffer pool, same bytes)
halves the final max8 latency in the t-dependency (3.4 us) and smooths
release pacing; 3-deep r/sel stream pools decouple the ACT/DVE producers
from PE matmul drain (2-deep pools lockstepped every stream op behind the
slowest consumer). Measured tile period 43-45 us = the DVE-bound floor.

Per-core dataflow, per 128-row s-tile (16 tiles), chunks of 3200 (10/tile):
  - DMA x chunks [128, 3200] f32 to SBUF (pool of 13), all on the sync
    HWDGE ring (a second ring — SWDGE or ACT — interleaves packets of two
    chunks and doubles per-chunk latency: measured ~25% regression)
  - DVE max8 per chunk -> concat [128,80] -> max8 -> t (8th largest)
  - value stream r = Relu(x - t) in fp16 (ACT), ones-stationary fp16
  - selection stream, split to balance DVE/ACT:
      * chunks 0-4 on DVE: sel = (x >= t) * 2 in {0,2}, fp16-exact (fused
        tensor_scalar 2x); contributes t/2 * 2m = t*m via w2
      * chunks 5-9 on ACT: sel = Sign(x - t') in {-1,+1}; t' = t*(1-2^-23)
        sits strictly between the 9th and 8th largest for every row of
        this input; the class-independent -t/2 is added back on the host
        (T_half = per-tile sum of fp16(t/2), acc2 psum column)
  - LAST tile: the post-final-byte serial tail is split for latency: DVE
    (2x tensor_scalar) takes all 10 sel chunks + relu of chunks 0-1, ACT
    takes relu of chunks 2-9. No sign stream in tile 15, so the host
    correction sums only tiles 0..14.
  - PE: per 400-wide class window, two matmuls accumulate into PSUM
    [80,400] via shifted one-hot stationaries (2560 total)
  - drain psum -> SBUF -> DRAM out [80,400] + tsum [1,16]; host adds
    T_half to the sign-path classes (>= 16000) and argmaxes.
"""

import sys

if "/opt/trn_rl_repo" not in sys.path:
    sys.path.insert(0, "/opt/trn_rl_repo")

import numpy as np

B, S, C = 8, 2048, 32000
TOP_K = 8
P = 128            # partitions per s-tile
XCH = 3200         # x chunk width
NXCH = C // XCH    # 10 chunks per tile
CCH = 400          # matmul moving window / psum columns
NW = XCH // CCH    # 8 windows per chunk
NROWS = C // CCH   # 80 psum rows
NTILES = S // P    # 16
# chunks whose selection runs on DVE in tiles 0..14 (rest: ACT Sign);
# classes below SIGN_CLASS_START never see the sign stream
DVE_SEL_CHUNKS = {0, 1, 2, 3, 4}
SIGN_CLASS_START = 5 * XCH  # 16000
# relu chunks of the LAST tile that run on DVE (ACT takes the rest)
TAIL_DVE_RELU = {0, 1}

_CACHE = {}


def _build_graph():
    from concourse import bacc, tile, mybir

    f32 = mybir.dt.float32
    f16 = mybir.dt.float16
    Alu = mybir.AluOpType
    Act = mybir.ActivationFunctionType

    nc = bacc.Bacc(
        "TRN2",
        target_bir_lowering=False,
        debug=False,
        # largest DMA here is 128 descriptors; the 16 KB default wastes SBUF
        dynamic_dma_scratch_size=4096,
    )
    x = nc.dram_tensor("x", [S, C], f32, kind="ExternalInput").ap()
    out = nc.dram_tensor("out", [NROWS, CCH], f32, kind="ExternalOutput").ap()
    tsum = nc.dram_tensor("tsum", [1, NTILES], f32, kind="ExternalOutput").ap()

    n_mm = NTILES * NXCH * NW * 2
    mm_i = 0
    SCALE_P = float(np.float32(1.0) - np.float32(2.0**-23))

    with tile.TileContext(nc) as tc:
        with (
            tc.tile_pool(name="consts", bufs=1) as consts,
            tc.tile_pool(name="xp", bufs=13) as xp,
            tc.tile_pool(name="tp", bufs=2) as tp,
            tc.tile_pool(name="rp", bufs=3) as rp,
            tc.tile_pool(name="selp", bufs=3) as selp,
            tc.tile_pool(name="sump", bufs=1) as sump,
            tc.tile_pool(name="ps", bufs=1, space="PSUM") as ps,
            tc.tile_pool(name="ps2", bufs=1, space="PSUM") as ps2,
        ):
            # ones stationary (relu stream + T_half column): 1.0 at col NROWS
            ztm = consts.tile([P, 2 * NROWS], f16, name="ztm")
            nc.vector.memset(ztm, 0.0)
            nc.vector.memset(ztm[:, NROWS : NROWS + 1], 1.0)
            # t/2 stationaries: zeros except col NROWS, overwritten with
            # fp16(t/2) every tile (parity double-buffered)
            w2 = [
                consts.tile([P, 2 * NROWS], f16, name=f"w2_{k}")
                for k in range(2)
            ]
            for w in w2:
                nc.vector.memset(w, 0.0)

            acc = ps.tile([NROWS, CCH], f32, name="acc")
            acc2 = ps2.tile([1, NTILES], f32, name="acc2")

            for it in range(NTILES):
                last = it == NTILES - 1
                xch = []
                for j in range(NXCH):
                    xt = xp.tile([P, XCH], f32, name="xch", tag="xch")
                    nc.sync.dma_start(
                        out=xt,
                        in_=x[it * P : (it + 1) * P, j * XCH : (j + 1) * XCH],
                    )
                    xch.append(xt)

                top = tp.tile([P, 8 * NXCH], f32, name="top", tag="top")
                for j in range(NXCH):
                    nc.vector.max(out=top[:, 8 * j : 8 * (j + 1)], in_=xch[j])
                top8 = tp.tile([P, 8], f32, name="top8", tag="top8")
                nc.vector.max(out=top8, in_=top)
                t_ap = top8[:, 7:8]

                # per-tile scalars: relu bias, sign bias, t/2 column
                tneg = tp.tile([P, 1], f32, name="tneg", tag="tneg")
                nc.vector.tensor_scalar(tneg, t_ap, -1.0, None, Alu.mult)
                if not last:
                    tpneg = tp.tile([P, 1], f32, name="tpneg", tag="tpneg")
                    nc.scalar.activation(
                        out=tpneg, in_=t_ap, func=Act.Copy, scale=-SCALE_P
                    )
                t2col = w2[it % 2][:, NROWS : NROWS + 1]
                nc.scalar.activation(
                    out=t2col, in_=t_ap, func=Act.Copy, scale=0.5
                )
                # per-tile T_half = sum_s fp16(t_s/2) (own psum column,
                # host-summed over tiles 0..14; reads the ROUNDED fp16 t/2
                # so the host correction matches the stationary exactly)
                t2f = tp.tile([P, 1], f16, name="t2f", tag="t2f")
                nc.scalar.activation(out=t2f, in_=t2col, func=Act.Copy)
                nc.tensor.matmul(
                    acc2[:, it : it + 1],
                    ztm[:, NROWS : NROWS + 1],
                    t2f,
                    start=True,
                    stop=True,
                )

                for j in range(NXCH):
                    xs = xch[j]
                    r = rp.tile([P, XCH], f16, name="r", tag="r")
                    if last and j in TAIL_DVE_RELU:
                        nc.vector.tensor_scalar(
                            r, xs, t_ap, 0.0, Alu.subtract, Alu.max
                        )
                    else:
                        nc.scalar.activation(
                            out=r,
                            in_=xs,
                            func=Act.Relu,
                            bias=tneg,
                            scale=1.0,
                        )
                    sel = selp.tile([P, XCH], f16, name="sel", tag="sel")
                    if last or j in DVE_SEL_CHUNKS:
                        nc.vector.tensor_scalar(
                            sel, xs, t_ap, 2.0, Alu.is_ge, Alu.mult
                        )
                    else:
                        nc.scalar.activation(
                            out=sel,
                            in_=xs,
                            func=Act.Sign,
                            bias=tpneg,
                            scale=1.0,
                        )
                    wsel = w2[it % 2]
                    for w in range(NW):
                        cg = j * NW + w
                        off = w * CCH
                        nc.tensor.matmul(
                            acc,
                            ztm[:, NROWS - cg : 2 * NROWS - cg],
                            r[:, off : off + CCH],
                            start=(mm_i == 0),
                            stop=(mm_i == n_mm - 1),
                        )
                        mm_i += 1
                        nc.tensor.matmul(
                            acc,
                            wsel[:, NROWS - cg : 2 * NROWS - cg],
                            sel[:, off : off + CCH],
                            start=False,
                            stop=(mm_i == n_mm - 1),
                        )
                        mm_i += 1

            sums = sump.tile([NROWS, CCH], f32, name="sums")
            nc.vector.tensor_copy(sums, acc)
            nc.sync.dma_start(out=out, in_=sums)
            tsums = sump.tile([1, NTILES], f32, name="tsums")
            nc.vector.tensor_copy(tsums, acc2)
            nc.sync.dma_start(out=tsum, in_=tsums)

    nc.compile()
    return nc


def _in_maps(x):
    return [{"x": np.ascontiguousarray(x[b])} for b in range(B)]


def _postprocess(sums_2d, tsum_row):
    sums = np.asarray(sums_2d, dtype=np.float64).reshape(-1)
    # tile 15 has no sign stream; only tiles 0..14 need the correction
    t_half = float(
        np.asarray(tsum_row, dtype=np.float64).reshape(-1)[: NTILES - 1].sum()
    )
    sums[SIGN_CLASS_START:] += t_half
    return sums


def kernel(**inputs):
    from concourse import bass_utils

    x = np.asarray(inputs["inputs"], dtype=np.float32)
    assert x.shape == (B, S, C), x.shape

    if "nc" not in _CACHE:
        _CACHE["nc"] = _build_graph()
    nc = _CACHE["nc"]

    res = bass_utils.run_bass_kernel_spmd(
        nc, _in_maps(x), core_ids=list(range(B))
    )

    out = np.empty((B,), dtype=np.int32)
    for b in range(B):
        sums = _postprocess(res.results[b]["out"], res.results[b]["tsum"])
        out[b] = np.argmax(sums)
    return out
